# revision 12
# baseline (speedup 1.0000x reference)
"""Trainium2 Bass kernel for nn_AttentionBlock (dense_cnn).

Computes, per batch b:
    a = sigmoid(MLP(x))              # per-pixel 2048->64->16->8->1 w/ ReLU
    out[b] = sum_p(a*x) / sum_p(a)   # weighted GAP over 14x14 pixels

Sharding: pure data parallelism over batch (B=64) across 8 NeuronCores
(8 batches/core); weights replicated; no cross-core communication.

Per-core layout strategy:
  - x shard viewed as [1568, 2048] (pixel-major, channel-minor).
  - Pixel tiles of 128 partitions; channel chunks of 128.
  - x tiles are PE-transposed on chip (chunk-wise) into xT [c, p] so the
    2048-contraction matmul can run; the MLP chain is kept transposed
    (channels on partitions) so ReLU+bias fuse into the PSUM->SBUF copy
    on the scalar engine and no per-layer transposes are needed.
  - GAP runs as a PE matmul with stationary masked-A [pix, 8 batches] and
    the *natural-layout* x tile as the moving operand, accumulating
    [8, 2048] in PSUM across all pixel tiles.  mean/mean == sum/sum.

All PE operand tensors are float32r (full-rate PE streaming at reduced
multiply precision); PSUM accumulation stays fp32.  Set USE_F32R = False
to fall back to exact (4x slower PE) float32.
"""

import numpy as np
from contextlib import ExitStack

from concourse import bacc, bass, mybir, tile
from concourse.bass_utils import run_bass_kernel_spmd

F32 = mybir.dt.float32
AF = mybir.ActivationFunctionType

USE_F32R = True
DT = mybir.dt.float32r if USE_F32R else F32

B, HH, WW, C = 64, 14, 14, 2048
NCORES = 8
BPC = B // NCORES            # 8 batches per core
PIX = HH * WW                # 196 pixels per batch
NPIX = BPC * PIX             # 1568 pixels per core
P = 128
NCH = C // P                 # 16 channel chunks
D1, D2, D3 = 64, 16, 8

# flat pixel tiles (DMA + GAP granularity)
TILES = [(t * P, min(P, NPIX - t * P)) for t in range((NPIX + P - 1) // P)]
NT = len(TILES)
# super-tiles (MLP chain granularity): pairs of pixel tiles -> 256-wide
# moving operands (needed for float32r full-rate)
SUPER = [[2 * s, 2 * s + 1] for s in range(NT // 2)]
if NT % 2:
    SUPER.append([NT - 1])


def build_program(b4_val: float):
    nc = bacc.Bacc("TRN2", target_bir_lowering=False, debug=False)

    x_d = nc.dram_tensor("x", [NPIX, C], DT, kind="ExternalInput")
    w1_d = nc.dram_tensor("W1r", [P, NCH, D1], DT, kind="ExternalInput")
    w2_d = nc.dram_tensor("W2", [D1, D2], DT, kind="ExternalInput")
    w3_d = nc.dram_tensor("W3", [D2, D3], DT, kind="ExternalInput")
    w4_d = nc.dram_tensor("W4", [D3, 2], DT, kind="ExternalInput")
    b1_d = nc.dram_tensor("b1c", [D1, 1], F32, kind="ExternalInput")
    b2_d = nc.dram_tensor("b2c", [D2, 1], F32, kind="ExternalInput")
    b3_d = nc.dram_tensor("b3c", [D3, 1], F32, kind="ExternalInput")
    id_d = nc.dram_tensor("ident", [P, P], DT, kind="ExternalInput")
    one_d = nc.dram_tensor("ones", [P, 2], DT, kind="ExternalInput")
    msk_d = nc.dram_tensor("mask", [P, NT, BPC], DT, kind="ExternalInput")
    out_d = nc.dram_tensor("out", [BPC, C], F32, kind="ExternalOutput")

    with tile.TileContext(nc) as tc, ExitStack() as ctx:
        const = ctx.enter_context(tc.tile_pool(name="const", bufs=1))
        acc = ctx.enter_context(tc.tile_pool(name="acc", bufs=1))
        xpool = ctx.enter_context(tc.tile_pool(name="xin", bufs=4))
        xtp = ctx.enter_context(tc.tile_pool(name="xT", bufs=2))
        hpool = ctx.enter_context(tc.tile_pool(name="hsb", bufs=2))
        misc = ctx.enter_context(tc.tile_pool(name="misc", bufs=3))
        ps_stage = ctx.enter_context(tc.tile_pool(name="stage", bufs=2, space="PSUM"))
        ps_chain = ctx.enter_context(tc.tile_pool(name="chain", bufs=2, space="PSUM"))
        ps_gap = ctx.enter_context(tc.tile_pool(name="gap", bufs=1, space="PSUM"))

        # ---- constants ----
        w1_sb = const.tile([P, NCH, D1], DT)
        nc.sync.dma_start(w1_sb[:], w1_d[:])
        w2_sb = const.tile([D1, D2], DT)
        nc.sync.dma_start(w2_sb[:], w2_d[:])
        w3_sb = const.tile([D2, D3], DT)
        nc.sync.dma_start(w3_sb[:], w3_d[:])
        w4_sb = const.tile([D3, 2], DT)
        nc.sync.dma_start(w4_sb[:], w4_d[:])
        b1_sb = const.tile([D1, 1], F32)
        nc.sync.dma_start(b1_sb[:], b1_d[:])
        b2_sb = const.tile([D2, 1], F32)
        nc.sync.dma_start(b2_sb[:], b2_d[:])
        b3_sb = const.tile([D3, 1], F32)
        nc.sync.dma_start(b3_sb[:], b3_d[:])
        ident = const.tile([P, P], DT)
        nc.sync.dma_start(ident[:], id_d[:])
        ones = const.tile([P, 2], DT)
        nc.sync.dma_start(ones[:], one_d[:])
        mask = const.tile([P, NT, BPC], DT)
        nc.sync.dma_start(mask[:], msk_d[:])

        # ---- accumulators ----
        gap_ps = ps_gap.tile([BPC, 4, 512], F32)      # [8, 2048] over 4 banks
        cnt_sb = acc.tile([BPC, 1], F32)
        nc.vector.memset(cnt_sb[:], 0.0)

        last_t = NT - 1
        gi = 0

        for tlist in SUPER:
            sz_list = [TILES[t][1] for t in tlist]
            s_sz = sum(sz_list)

            # ---- load x tiles (natural layout) ----
            xts = []
            for t in tlist:
                off, sz = TILES[t]
                xt = xpool.tile([sz, C], DT, tag="x")
                nc.sync.dma_start(xt[:], x_d[off:off + sz, :])
                xts.append(xt)

            # ---- transpose x -> xT [c, pix] chunk-wise via PE ----
            xT = xtp.tile([P, NCH, s_sz], DT, tag="xT")
            xT_flat = xT[:].rearrange("p k s -> p (k s)")
            pairs = []          # (k, i, xT-linear offset, width)
            loff = 0
            for k in range(NCH):
                for i in range(len(tlist)):
                    pairs.append((k, i, loff, sz_list[i]))
                    loff += sz_list[i]
            # pack transposes into 512-wide PSUM staging groups
            g0 = 0
            while g0 < len(pairs):
                g1 = g0
                gw = 0
                while g1 < len(pairs) and gw + pairs[g1][3] <= 512:
                    gw += pairs[g1][3]
                    g1 += 1
                stage = ps_stage.tile([P, 512], DT, tag="stage")
                goff = pairs[g0][2]
                for (k, i, loff, w) in pairs[g0:g1]:
                    nc.tensor.transpose(
                        stage[:, loff - goff:loff - goff + w],
                        xts[i][:, k * P:(k + 1) * P],
                        ident[0:sz_list[i], 0:sz_list[i]],
                    )
                # split PSUM->SBUF copies between DVE and ACT
                dst = xT_flat[:, goff:goff + gw]
                if gi % 2 == 0:
                    nc.vector.tensor_copy(dst, stage[:, 0:gw])
                else:
                    nc.scalar.activation(dst, stage[:, 0:gw], AF.Copy)
                gi += 1
                g0 = g1

            # ---- transposed MLP chain (channels on partitions) ----
            h1_ps = ps_chain.tile([D1, s_sz], F32, tag="chain")
            for k in range(NCH):
                nc.tensor.matmul(
                    h1_ps[:], w1_sb[:, k, :], xT[:, k, :],
                    start=(k == 0), stop=(k == NCH - 1),
                )
            h1_sb = hpool.tile([D1, s_sz], DT, tag="h1")
            nc.scalar.activation(h1_sb[:], h1_ps[:], AF.Relu, bias=b1_sb[:])

            h2_ps = ps_chain.tile([D2, s_sz], F32, tag="chain")
            nc.tensor.matmul(h2_ps[:], w2_sb[:], h1_sb[:], start=True, stop=True)
            h2_sb = hpool.tile([D2, s_sz], DT, tag="h2")
            nc.scalar.activation(h2_sb[:], h2_ps[:], AF.Relu, bias=b2_sb[:])

            h3_ps = ps_chain.tile([D3, s_sz], F32, tag="chain")
            nc.tensor.matmul(h3_ps[:], w3_sb[:], h2_sb[:], start=True, stop=True)
            h3_sb = hpool.tile([D3, s_sz], DT, tag="h3")
            nc.scalar.activation(h3_sb[:], h3_ps[:], AF.Relu, bias=b3_sb[:])

            # ---- per pixel-tile: attention column, mask, GAP ----
            for i, t in enumerate(tlist):
                off, sz = TILES[t]
                i0 = i * P
                a_ps = ps_chain.tile([sz, 2], F32, tag="chain")
                nc.tensor.matmul(a_ps[:], h3_sb[:, i0:i0 + sz], w4_sb[:],
                                 start=True, stop=True)
                a_sb = misc.tile([sz, 1], DT, tag="a")
                nc.scalar.activation(a_sb[:], a_ps[:, 0:1], AF.Sigmoid, bias=b4_val)

                A = misc.tile([sz, BPC], DT, tag="A")
                nc.vector.tensor_mul(A[:], a_sb[:].to_broadcast([sz, BPC]),
                                     mask[0:sz, t, :])

                for n in range(4):
                    nc.tensor.matmul(
                        gap_ps[:, n, :], A[:],
                        xts[i][:, n * 512:(n + 1) * 512],
                        start=(t == 0), stop=(t == last_t),
                    )
                cnt_ps = ps_chain.tile([BPC, 2], F32, tag="chain")
                nc.tensor.matmul(cnt_ps[:], A[:], ones[0:sz, :],
                                 start=True, stop=True)
                nc.vector.tensor_add(cnt_sb[:], cnt_sb[:], cnt_ps[:, 0:1])

        # ---- finalize: out = gap_sum / cnt ----
        recip = acc.tile([BPC, 1], F32)
        nc.vector.reciprocal(recip[:], cnt_sb[:])
        out_sb = acc.tile([BPC, C], F32)
        for n in range(4):
            nc.scalar.activation(out_sb[:, n * 512:(n + 1) * 512],
                                 gap_ps[:, n, :], AF.Copy, scale=recip[:])
        nc.sync.dma_start(out_d[:], out_sb[:])

    nc.compile()
    return nc


def _make_mask():
    m = np.zeros((P, NT, BPC), dtype=np.float32)
    for t, (off, sz) in enumerate(TILES):
        for p in range(sz):
            m[p, t, (off + p) // PIX] = 1.0
    return m


def make_in_maps(x, W1, b1, W2, b2, W3, b3, W4, b4):
    x = np.ascontiguousarray(np.asarray(x, dtype=np.float32))
    base = {
        "W1r": np.ascontiguousarray(
            np.asarray(W1, np.float32).reshape(NCH, P, D1).transpose(1, 0, 2)),
        "W2": np.ascontiguousarray(np.asarray(W2, np.float32)),
        "W3": np.ascontiguousarray(np.asarray(W3, np.float32)),
        "W4": np.ascontiguousarray(np.concatenate(
            [np.asarray(W4, np.float32),
             np.zeros((D3, 1), np.float32)], axis=1)),
        "b1c": np.asarray(b1, np.float32).reshape(D1, 1).copy(),
        "b2c": np.asarray(b2, np.float32).reshape(D2, 1).copy(),
        "b3c": np.asarray(b3, np.float32).reshape(D3, 1).copy(),
        "ident": np.eye(P, dtype=np.float32),
        "ones": np.ones((P, 2), dtype=np.float32),
        "mask": _make_mask(),
    }
    xs = x.reshape(B, PIX, C)
    return [
        {"x": np.ascontiguousarray(xs[c * BPC:(c + 1) * BPC].reshape(NPIX, C)),
         **base}
        for c in range(NCORES)
    ]


def kernel(x, W1, b1, W2, b2, W3, b3, W4, b4, _profile=False, **_ignored):
    nc = build_program(float(np.asarray(b4, np.float32).reshape(-1)[0]))
    in_maps = make_in_maps(x, W1, b1, W2, b2, W3, b3, W4, b4)
    res = run_bass_kernel_spmd(nc, in_maps, core_ids=list(range(NCORES)),
                               trace=_profile)
    out = np.concatenate([res.results[c]["out"] for c in range(NCORES)], axis=0)
    out = np.ascontiguousarray(out.astype(np.float32))
    if _profile:
        return out, res
    return out


# revision 13
# speedup vs baseline: 1.1389x; 1.1389x over previous
"""Trainium2 Bass kernel for nn_AttentionBlock (dense_cnn).

Computes, per batch b:
    a = sigmoid(MLP(x))              # per-pixel 2048->64->16->8->1 w/ ReLU
    out[b] = sum_p(a*x) / sum_p(a)   # weighted GAP over 14x14 pixels

Sharding: pure data parallelism over batch (B=64) across 8 NeuronCores
(8 batches/core); weights replicated; no cross-core communication.

Per-core layout strategy:
  - x shard viewed as [1568, 2048] (pixel-major, channel-minor).
  - Pixel tiles of 128 partitions; channel chunks of 128.
  - x tiles are PE-transposed on chip (chunk-wise) into xT [c, p] so the
    2048-contraction matmul can run; the MLP chain is kept transposed
    (channels on partitions) so ReLU+bias fuse into the PSUM->SBUF copy
    on the scalar engine and no per-layer transposes are needed.
  - GAP runs as a PE matmul with stationary masked-A [pix, 8 batches] and
    the *natural-layout* x tile as the moving operand, accumulating
    [8, 2048] in PSUM across all pixel tiles.  mean/mean == sum/sum.

All PE operand tensors are float32r (full-rate PE streaming at reduced
multiply precision); PSUM accumulation stays fp32.  Set USE_F32R = False
to fall back to exact (4x slower PE) float32.
"""

import ml_dtypes
import numpy as np
from contextlib import ExitStack

from concourse import bacc, bass, mybir, tile
from concourse.bass_utils import run_bass_kernel_spmd

F32 = mybir.dt.float32
AF = mybir.ActivationFunctionType

USE_F32R = True
DT = mybir.dt.float32r if USE_F32R else F32      # GAP path (output-critical)
BF = mybir.dt.bfloat16                           # transpose + MLP chain path

B, HH, WW, C = 64, 14, 14, 2048
NCORES = 8
BPC = B // NCORES            # 8 batches per core
PIX = HH * WW                # 196 pixels per batch
NPIX = BPC * PIX             # 1568 pixels per core
P = 128
NCH = C // P                 # 16 channel chunks
D1, D2, D3 = 64, 16, 8

# flat pixel tiles (DMA + GAP granularity)
TILES = [(t * P, min(P, NPIX - t * P)) for t in range((NPIX + P - 1) // P)]
NT = len(TILES)
# super-tiles (MLP chain granularity): pairs of pixel tiles -> 256-wide
# moving operands (needed for float32r full-rate)
SUPER = [[2 * s, 2 * s + 1] for s in range(NT // 2)]
if NT % 2:
    SUPER.append([NT - 1])


def build_program(b4_val: float):
    nc = bacc.Bacc("TRN2", target_bir_lowering=False, debug=False)

    x_d = nc.dram_tensor("x", [NPIX, C], DT, kind="ExternalInput")
    xb_d = nc.dram_tensor("xb", [NPIX, C], BF, kind="ExternalInput")
    w1_d = nc.dram_tensor("W1r", [P, NCH, D1], BF, kind="ExternalInput")
    w2_d = nc.dram_tensor("W2", [D1, D2], BF, kind="ExternalInput")
    w3_d = nc.dram_tensor("W3", [D2, D3], BF, kind="ExternalInput")
    w4_d = nc.dram_tensor("W4", [D3, 2], BF, kind="ExternalInput")
    b1_d = nc.dram_tensor("b1c", [D1, 1], F32, kind="ExternalInput")
    b2_d = nc.dram_tensor("b2c", [D2, 1], F32, kind="ExternalInput")
    b3_d = nc.dram_tensor("b3c", [D3, 1], F32, kind="ExternalInput")
    id_d = nc.dram_tensor("ident", [P, P], BF, kind="ExternalInput")
    one_d = nc.dram_tensor("ones", [P, 2], DT, kind="ExternalInput")
    msk_d = nc.dram_tensor("mask", [P, NT, BPC], DT, kind="ExternalInput")
    out_d = nc.dram_tensor("out", [BPC, C], F32, kind="ExternalOutput")

    with tile.TileContext(nc) as tc, ExitStack() as ctx:
        const = ctx.enter_context(tc.tile_pool(name="const", bufs=1))
        acc = ctx.enter_context(tc.tile_pool(name="acc", bufs=1))
        xpool = ctx.enter_context(tc.tile_pool(name="xin", bufs=4))
        xbpool = ctx.enter_context(tc.tile_pool(name="xbin", bufs=4))
        xtp = ctx.enter_context(tc.tile_pool(name="xT", bufs=2))
        hpool = ctx.enter_context(tc.tile_pool(name="hsb", bufs=2))
        misc = ctx.enter_context(tc.tile_pool(name="misc", bufs=3))
        ps_stage = ctx.enter_context(tc.tile_pool(name="stage", bufs=2, space="PSUM"))
        ps_chain = ctx.enter_context(tc.tile_pool(name="chain", bufs=2, space="PSUM"))
        ps_gap = ctx.enter_context(tc.tile_pool(name="gap", bufs=1, space="PSUM"))

        # ---- constants ----
        w1_sb = const.tile([P, NCH, D1], BF)
        nc.sync.dma_start(w1_sb[:], w1_d[:])
        w2_sb = const.tile([D1, D2], BF)
        nc.sync.dma_start(w2_sb[:], w2_d[:])
        w3_sb = const.tile([D2, D3], BF)
        nc.sync.dma_start(w3_sb[:], w3_d[:])
        w4_sb = const.tile([D3, 2], BF)
        nc.sync.dma_start(w4_sb[:], w4_d[:])
        b1_sb = const.tile([D1, 1], F32)
        nc.sync.dma_start(b1_sb[:], b1_d[:])
        b2_sb = const.tile([D2, 1], F32)
        nc.sync.dma_start(b2_sb[:], b2_d[:])
        b3_sb = const.tile([D3, 1], F32)
        nc.sync.dma_start(b3_sb[:], b3_d[:])
        ident = const.tile([P, P], BF)
        nc.sync.dma_start(ident[:], id_d[:])
        ones = const.tile([P, 2], DT)
        nc.sync.dma_start(ones[:], one_d[:])
        mask = const.tile([P, NT, BPC], DT)
        nc.sync.dma_start(mask[:], msk_d[:])

        # ---- accumulators ----
        gap_ps = ps_gap.tile([BPC, 4, 512], F32)      # [8, 2048] over 4 banks
        cnt_sb = acc.tile([BPC, 1], F32)
        nc.vector.memset(cnt_sb[:], 0.0)

        last_t = NT - 1
        gi = 0

        for tlist in SUPER:
            sz_list = [TILES[t][1] for t in tlist]
            s_sz = sum(sz_list)

            # ---- load x tiles (natural layout) ----
            xts = []
            xbs = []
            for t in tlist:
                off, sz = TILES[t]
                xt = xpool.tile([sz, C], DT, tag="x")
                nc.sync.dma_start(xt[:], x_d[off:off + sz, :])
                xts.append(xt)
                xbt = xbpool.tile([sz, C], BF, tag="xb")
                nc.scalar.dma_start(xbt[:], xb_d[off:off + sz, :])
                xbs.append(xbt)

            # ---- transpose x -> xT [c, pix] chunk-wise via PE ----
            xT = xtp.tile([P, NCH, s_sz], BF, tag="xT")
            xT_flat = xT[:].rearrange("p k s -> p (k s)")
            pairs = []          # (k, i, xT-linear offset, width)
            loff = 0
            for k in range(NCH):
                for i in range(len(tlist)):
                    pairs.append((k, i, loff, sz_list[i]))
                    loff += sz_list[i]
            # pack transposes into 512-wide PSUM staging groups
            g0 = 0
            while g0 < len(pairs):
                g1 = g0
                gw = 0
                while g1 < len(pairs) and gw + pairs[g1][3] <= 512:
                    gw += pairs[g1][3]
                    g1 += 1
                stage = ps_stage.tile([P, 512], BF, tag="stage")
                goff = pairs[g0][2]
                for (k, i, loff, w) in pairs[g0:g1]:
                    nc.tensor.transpose(
                        stage[:, loff - goff:loff - goff + w],
                        xbs[i][:, k * P:(k + 1) * P],
                        ident[0:sz_list[i], 0:sz_list[i]],
                    )
                # split PSUM->SBUF copies between DVE and ACT
                dst = xT_flat[:, goff:goff + gw]
                if gi % 2 == 0:
                    nc.vector.tensor_copy(dst, stage[:, 0:gw])
                else:
                    nc.scalar.activation(dst, stage[:, 0:gw], AF.Copy)
                gi += 1
                g0 = g1

            # ---- transposed MLP chain (channels on partitions) ----
            h1_ps = ps_chain.tile([D1, s_sz], F32, tag="chain")
            for k in range(NCH):
                nc.tensor.matmul(
                    h1_ps[:], w1_sb[:, k, :], xT[:, k, :],
                    start=(k == 0), stop=(k == NCH - 1),
                )
            h1_sb = hpool.tile([D1, s_sz], BF, tag="h1")
            nc.scalar.activation(h1_sb[:], h1_ps[:], AF.Relu, bias=b1_sb[:])

            h2_ps = ps_chain.tile([D2, s_sz], F32, tag="chain")
            nc.tensor.matmul(h2_ps[:], w2_sb[:], h1_sb[:], start=True, stop=True)
            h2_sb = hpool.tile([D2, s_sz], BF, tag="h2")
            nc.scalar.activation(h2_sb[:], h2_ps[:], AF.Relu, bias=b2_sb[:])

            h3_ps = ps_chain.tile([D3, s_sz], F32, tag="chain")
            nc.tensor.matmul(h3_ps[:], w3_sb[:], h2_sb[:], start=True, stop=True)
            h3_sb = hpool.tile([D3, s_sz], BF, tag="h3")
            nc.scalar.activation(h3_sb[:], h3_ps[:], AF.Relu, bias=b3_sb[:])

            # ---- per pixel-tile: attention column, mask, GAP ----
            for i, t in enumerate(tlist):
                off, sz = TILES[t]
                i0 = i * P
                a_ps = ps_chain.tile([sz, 2], F32, tag="chain")
                nc.tensor.matmul(a_ps[:], h3_sb[:, i0:i0 + sz], w4_sb[:],
                                 start=True, stop=True)
                a_sb = misc.tile([sz, 1], DT, tag="a")
                nc.scalar.activation(a_sb[:], a_ps[:, 0:1], AF.Sigmoid, bias=b4_val)

                A = misc.tile([sz, BPC], DT, tag="A")
                nc.vector.tensor_mul(A[:], a_sb[:].to_broadcast([sz, BPC]),
                                     mask[0:sz, t, :])

                for n in range(4):
                    nc.tensor.matmul(
                        gap_ps[:, n, :], A[:],
                        xts[i][:, n * 512:(n + 1) * 512],
                        start=(t == 0), stop=(t == last_t),
                    )
                cnt_ps = ps_chain.tile([BPC, 2], F32, tag="chain")
                nc.tensor.matmul(cnt_ps[:], A[:], ones[0:sz, :],
                                 start=True, stop=True)
                nc.vector.tensor_add(cnt_sb[:], cnt_sb[:], cnt_ps[:, 0:1])

        # ---- finalize: out = gap_sum / cnt ----
        recip = acc.tile([BPC, 1], F32)
        nc.vector.reciprocal(recip[:], cnt_sb[:])
        out_sb = acc.tile([BPC, C], F32)
        for n in range(4):
            nc.scalar.activation(out_sb[:, n * 512:(n + 1) * 512],
                                 gap_ps[:, n, :], AF.Copy, scale=recip[:])
        nc.sync.dma_start(out_d[:], out_sb[:])

    nc.compile()
    return nc


def _make_mask():
    m = np.zeros((P, NT, BPC), dtype=np.float32)
    for t, (off, sz) in enumerate(TILES):
        for p in range(sz):
            m[p, t, (off + p) // PIX] = 1.0
    return m


def make_in_maps(x, W1, b1, W2, b2, W3, b3, W4, b4):
    x = np.ascontiguousarray(np.asarray(x, dtype=np.float32))
    base = {
        "W1r": np.ascontiguousarray(
            np.asarray(W1, np.float32).reshape(NCH, P, D1).transpose(1, 0, 2)
            .astype(ml_dtypes.bfloat16)),
        "W2": np.ascontiguousarray(np.asarray(W2, ml_dtypes.bfloat16)),
        "W3": np.ascontiguousarray(np.asarray(W3, ml_dtypes.bfloat16)),
        "W4": np.ascontiguousarray(np.concatenate(
            [np.asarray(W4, np.float32),
             np.zeros((D3, 1), np.float32)], axis=1).astype(ml_dtypes.bfloat16)),
        "b1c": np.asarray(b1, np.float32).reshape(D1, 1).copy(),
        "b2c": np.asarray(b2, np.float32).reshape(D2, 1).copy(),
        "b3c": np.asarray(b3, np.float32).reshape(D3, 1).copy(),
        "ident": np.eye(P, dtype=ml_dtypes.bfloat16),
        "ones": np.ones((P, 2), dtype=np.float32),
        "mask": _make_mask(),
    }
    xs = x.reshape(B, PIX, C)
    maps = []
    for c in range(NCORES):
        xc = np.ascontiguousarray(xs[c * BPC:(c + 1) * BPC].reshape(NPIX, C))
        maps.append({"x": xc, "xb": xc.astype(ml_dtypes.bfloat16), **base})
    return maps


def kernel(x, W1, b1, W2, b2, W3, b3, W4, b4, _profile=False, **_ignored):
    nc = build_program(float(np.asarray(b4, np.float32).reshape(-1)[0]))
    in_maps = make_in_maps(x, W1, b1, W2, b2, W3, b3, W4, b4)
    res = run_bass_kernel_spmd(nc, in_maps, core_ids=list(range(NCORES)),
                               trace=_profile)
    out = np.concatenate([res.results[c]["out"] for c in range(NCORES)], axis=0)
    out = np.ascontiguousarray(out.astype(np.float32))
    if _profile:
        return out, res
    return out
